# revision 35
# baseline (speedup 1.0000x reference)
"""Trainium2 Bass kernel for a 4-step differentiable recurrent net forward pass.

Reference computation (B=8192, NI=512, NH=2048, NO=512, 4 steps):
    activs = 0; outputs = 0
    repeat 4x:  pre = hr * (x @ Wih.T + activs @ Whh.T + outputs @ Woh.T) + hb
                activs = per_neuron_act(pre)        # tanh/sigmoid/relu by i%3
    out = sigmoid(or * (x @ Wio.T + outputs @ Woo.T + activs @ Who.T) + ob)

`outputs` is never written inside the loop, so the Woh/Woo terms vanish and
the x-projection P = hr*(x@Wih.T)+hb is loop-invariant (computed once).

Strategy: data-parallel on batch across 8 cores (1024 rows each). On-core
everything is feature-major (features on SBUF partitions, batch on the free
axis). All matmuls run in fp8 e4m3 with DoubleRow perf mode (two 128-deep
k-tiles contracted per pass -> 2x tensor throughput vs bf16); weights are
host-quantized at a x256 power-of-2 scale (so typical |w|~5 stays in e4m3's
normal range), activations/x are quantized at unit scale, and PSUM holds
256x the true (biasless) pre-activations; the 1/256 and the true hb/ob are
folded into the scalar-engine activation `scale`/`bias` operands. The
x-projection P' is evicted once per 4-bank PSUM block with a single DVE
read; the output layer accumulates x@Wio into the same PSUM group as
A@Who and applies sigmoid straight off PSUM. DMA dispatch order is tuned
from the perfetto trace: both HWDGE queues run 4-deep semaphore rings, so
pieces are ordered by consumption time and whh streams in 512KB halves.
Host-side prep: hidden neurons are permuted so the three activation groups
are contiguous, hr/or are folded into the weights, and weights are packed
so each piece loads as one contiguous-row DMA.
"""

import os

import numpy as np
import ml_dtypes

import concourse.bass as bass
import concourse.tile as tile
from concourse import bacc, mybir
from concourse.bass_utils import run_bass_kernel_spmd

B, NI, NH, NO = 8192, 512, 2048, 512
N_STEPS = 4
N_CORES = 8
BL = B // N_CORES          # batch rows per core
CH = 512                   # batch chunk (one PSUM bank of fp32)
NCH = BL // CH             # 2 chunks per core
KI = NI // 128             # 4 k-tiles over inputs
KH = NH // 128             # 16 k/m-tiles over hidden
KO = NO // 128             # 4 m-tiles over outputs

BF16 = mybir.dt.bfloat16
F32 = mybir.dt.float32
FP8 = mybir.dt.float8e4
E4 = ml_dtypes.float8_e4m3   # TRN-style e4m3, max normal 240
AF = mybir.ActivationFunctionType
DR = mybir.MatmulPerfMode.DoubleRow

SW = 256.0                 # weight quantization scale (power of 2)
INV = 1.0 / SW             # folded into activation-engine `scale`

# hidden neurons regrouped as [all tanh | all sigmoid | all relu]
_idx = np.arange(NH)
PERM = np.concatenate([_idx[_idx % 3 == 0], _idx[_idx % 3 == 1], _idx[_idx % 3 == 2]])
_B1 = int((_idx % 3 == 0).sum())           # 683
_B2 = _B1 + int((_idx % 3 == 1).sum())     # 1366

# per m-tile: the single activation function, or None for the two mixed tiles
_TILE_FUNC = []
for _m in range(KH):
    _lo, _hi = _m * 128, (_m + 1) * 128
    _fs = set()
    for _f, _a, _b in ((AF.Tanh, 0, _B1), (AF.Sigmoid, _B1, _B2), (AF.Relu, _B2, NH)):
        if max(_lo, _a) < min(_hi, _b):
            _fs.add(_f)
    _TILE_FUNC.append(_fs.pop() if len(_fs) == 1 else None)

# mixed tiles: (major_func applied everywhere, minor_func, mask column block)
# partition sub-ranges must be 32-aligned on TRN2, so the minority strip is
# fixed up with a full-tile ACT + copy_predicated against a {0,1} mask
_BOUNDARY = {
    _B1 // 128: (AF.Sigmoid, AF.Tanh, 0),    # tile 5: parts < 43 are tanh
    _B2 // 128: (AF.Sigmoid, AF.Relu, 1),    # tile 10: parts >= 86 are relu
}


def _emit_hidden_act(nc, ps, blk, a_new, tmp_pool, bmask_t, hbc_t):
    """Evict a 4-bank pre-activation block through the grouped activations.

    ps:    AP (128, 4*CH) holding 256x-scaled biasless pre-activations for
           m-tiles blk*4..blk*4+3 (SBUF f32/bf16); the true hidden bias is
           applied per-tile via the ACT bias operand
    a_new: SBUF tile (128, KH, CH) fp8, m-tile m lives at [:, m, :]
    """
    for mloc in range(4):
        m = blk * 4 + mloc
        bias = hbc_t[:, m:m + 1]
        if m in _BOUNDARY:
            major, minor, mb = _BOUNDARY[m]
            nc.scalar.activation(
                a_new[:, m, :],
                ps[:, mloc * CH:(mloc + 1) * CH], major, bias=bias, scale=INV)
            t = tmp_pool.tile([128, CH], FP8, tag="btmp", bufs=2, name="btmp")
            nc.scalar.activation(t[:], ps[:, mloc * CH:(mloc + 1) * CH], minor,
                                 bias=bias, scale=INV)
            nc.vector.copy_predicated(
                a_new[:, m, :],
                bmask_t[:, mb * CH:(mb + 1) * CH], t[:])
        else:
            nc.scalar.activation(
                a_new[:, m, :],
                ps[:, mloc * CH:(mloc + 1) * CH], _TILE_FUNC[m],
                bias=bias, scale=INV)


def _build_nc():
    nc = bacc.Bacc("TRN2", target_bir_lowering=False, debug=False,
                   num_devices=N_CORES, dynamic_dma_scratch_size=2048)

    # all operands host-packed so each loads as one large contiguous DMA:
    # [p, k, c] = W[k*128 + p, c] for k-tile k; fp8 e4m3 at 256x scale
    xT = nc.dram_tensor("xT", [128, KI, BL], FP8, kind="ExternalInput").ap()
    wih = nc.dram_tensor("wih", [128, KI, NH], FP8, kind="ExternalInput").ap()
    whh = nc.dram_tensor("whh", [4 * 128, 4, NH], FP8,
                         kind="ExternalInput").ap()
    who = nc.dram_tensor("who", [128, KO * KO, NO], FP8,
                         kind="ExternalInput").ap()
    wio = nc.dram_tensor("wio", [128, KI, NO], FP8, kind="ExternalInput").ap()
    hbc = nc.dram_tensor("hbc", [128, KH], F32, kind="ExternalInput").ap()
    obc = nc.dram_tensor("obc", [128, KO], F32, kind="ExternalInput").ap()
    bmask = nc.dram_tensor("bmask", [128, 2 * CH], mybir.dt.uint8,
                           kind="ExternalInput").ap()
    outT = nc.dram_tensor("outT", [NO, BL], BF16, kind="ExternalOutput").ap()

    with tile.TileContext(nc) as tc:
        with tc.tile_pool(name="w", bufs=1) as wpool, \
             tc.tile_pool(name="act", bufs=1) as apool, \
             tc.tile_pool(name="ps", bufs=2, space="PSUM") as pspool, \
             tc.tile_pool(name="out", bufs=4) as opool:

            # ---- stage inputs. DMA dispatch order is tuned from the trace:
            # each HWDGE queue (SP=sync, ACT=scalar) runs a 4-deep semaphore
            # ring, so pieces are ordered by consumption time and whh is
            # split into 512KB halves so the rings stream continuously.
            # The very first pieces are exactly what the first matmuls need
            # (k-pair 0/1, m-tiles 0-3 + x chunk 0). ----
            wih_m = wpool.tile([128, KI, NH], FP8, tag="projA", name="wihm")
            x_m = wpool.tile([128, KI, BL], FP8, tag="x", name="xm")
            # SP queue: wih in P-loop consumption order, tiny per-partition
            # tensors, then whh J0/J2 halves
            nc.sync.dma_start(wih_m[:, 0:2, 0:4 * 128], wih[:, 0:2, 0:4 * 128])
            nc.sync.dma_start(wih_m[:, 2:4, 0:4 * 128], wih[:, 2:4, 0:4 * 128])
            nc.sync.dma_start(wih_m[:, 0:4, 4 * 128:8 * 128],
                              wih[:, 0:4, 4 * 128:8 * 128])
            hbc_t = wpool.tile([128, KH], F32, tag="hbc")
            nc.sync.dma_start(hbc_t[:], hbc[:])
            bmask_t = wpool.tile([128, 2 * CH], mybir.dt.uint8, tag="bmask")
            nc.sync.dma_start(bmask_t[:], bmask[:])
            obc_t = wpool.tile([128, KO], F32, tag="obc")
            nc.sync.dma_start(obc_t[:], obc[:])
            # ACT queue: x and the wih tail interleaved by consumption time,
            # then whh J1/J3 halves
            nc.scalar.dma_start(x_m[:, 0:2, 0:CH], xT[:, 0:2, 0:CH])
            nc.scalar.dma_start(x_m[:, 2:4, 0:CH], xT[:, 2:4, 0:CH])
            nc.scalar.dma_start(wih_m[:, 0:4, 8 * 128:12 * 128],
                                wih[:, 0:4, 8 * 128:12 * 128])
            nc.scalar.dma_start(x_m[:, 0:2, CH:BL], xT[:, 0:2, CH:BL])
            nc.scalar.dma_start(wih_m[:, 0:4, 12 * 128:NH],
                                wih[:, 0:4, 12 * 128:NH])
            nc.scalar.dma_start(x_m[:, 2:4, CH:BL], xT[:, 2:4, CH:BL])
            whh_m = [wpool.tile([128, 4, NH], FP8, tag=f"whhJ{J}",
                                name=f"whhJ{J}") for J in range(4)]
            for J, eng in ((0, nc.sync), (1, nc.scalar),
                           (2, nc.sync), (3, nc.scalar)):
                for h in range(2):
                    eng.dma_start(whh_m[J][:, 2 * h:2 * h + 2, :],
                                  whh[J * 128:(J + 1) * 128, 2 * h:2 * h + 2, :])

            # ---- per-chunk x-projection P' (256x scaled, biasless) and
            # first-step activations ----
            P = {}
            A = {}

            def p_chunk(c):
                P[c] = apool.tile([128, KH * CH], BF16, tag=f"P{c}",
                                  name=f"P{c}")
                a1 = apool.tile([128, KH, CH], FP8, tag="A", bufs=3,
                                name=f"A1c{c}")
                for blk in range(4):
                    ps = pspool.tile([128, 4 * CH], F32, tag="ps", name="psb")
                    for kp in range(KI // 2):
                        for mloc in range(4):
                            m = blk * 4 + mloc
                            nc.tensor.matmul(
                                ps[:, mloc * CH:(mloc + 1) * CH],
                                wih_m[:, 2 * kp:2 * kp + 2,
                                      m * 128:(m + 1) * 128],
                                x_m[:, 2 * kp:2 * kp + 2,
                                    c * CH:(c + 1) * CH],
                                start=(kp == 0), stop=(kp == KI // 2 - 1),
                                perf_mode=DR)
                    # single DVE read frees the 4-bank PSUM slot; ACTs then
                    # run off SBUF with the bias applied per tile
                    nc.vector.tensor_copy(
                        P[c][:, blk * 4 * CH:(blk + 1) * 4 * CH], ps[:])
                    _emit_hidden_act(nc, P[c][:, blk * 4 * CH:(blk + 1) * 4 * CH],
                                     blk, a1, opool, bmask_t, hbc_t)
                A[c] = a1

            p_chunk(0)
            p_chunk(1)

            # ---- recurrent steps 2..4 ----
            def hh_step(c, s):
                a_new = apool.tile([128, KH, CH], FP8, tag="A", bufs=3,
                                   name=f"A{s + 2}c{c}")
                for blk in range(4):
                    ps = pspool.tile([128, 4 * CH], F32, tag="ps", name="psb")
                    for kp in range(KH // 2):
                        J, sub = divmod(kp, 2)
                        for mloc in range(4):
                            m = blk * 4 + mloc
                            nc.tensor.matmul(
                                ps[:, mloc * CH:(mloc + 1) * CH],
                                whh_m[J][:, 2 * sub:2 * sub + 2,
                                         m * 128:(m + 1) * 128],
                                A[c][:, 2 * kp:2 * kp + 2, :],
                                start=(kp == 0), stop=(kp == KH // 2 - 1),
                                perf_mode=DR)
                    # pre' = psum + P' into an SBUF temp: a single PSUM read
                    # frees the bank; ACT then runs off SBUF
                    tmp = opool.tile([128, 4 * CH], F32, tag="pre", bufs=3,
                                     name="pre")
                    nc.vector.tensor_add(
                        tmp[:], ps[:], P[c][:, blk * 4 * CH:(blk + 1) * 4 * CH])
                    _emit_hidden_act(nc, tmp, blk, a_new, opool, bmask_t, hbc_t)
                A[c] = a_new

            for s in range(N_STEPS - 2):
                for c in range(NCH):
                    hh_step(c, s)
            hh_step(0, N_STEPS - 2)  # chunk 1's final step emitted after who

            # ---- output layer (who reuses the wih slot); chunk 0's
            # output overlaps chunk 1's final hh step. The x@Wio term is
            # accumulated into the same PSUM group, and sigmoid runs
            # directly off PSUM with bias+scale — no outx staging at all ----
            wio_m = wpool.tile([128, KI, NO], FP8, tag="wio", name="wiom")
            nc.sync.dma_start(wio_m[:], wio[:])
            who_m = wpool.tile([128, KO * KO, NO], FP8, tag="projA",
                               name="whom")
            nc.sync.dma_start(who_m[:, 0:8, :], who[:, 0:8, :])
            nc.sync.dma_start(who_m[:, 8:16, :], who[:, 8:16, :])

            def out_chunk(c):
                pso = None
                for mo in range(KO):
                    # two mo-tiles share one 4-bank PSUM tile: halves the
                    # pool rotation so mo N+2 never waits on mo N's sigmoids
                    if mo % 2 == 0:
                        pso = pspool.tile([128, 4 * CH], F32, tag="ps",
                                          name="pso")
                    oap = pso[:, (mo % 2) * CH:(mo % 2 + 1) * CH]
                    for t2 in range(KH // 2):
                        nc.tensor.matmul(
                            oap,
                            who_m[:, 2 * t2:2 * t2 + 2,
                                  mo * 128:(mo + 1) * 128],
                            A[c][:, 2 * t2:2 * t2 + 2, :],
                            start=(t2 == 0), stop=False, perf_mode=DR)
                    for kp in range(KI // 2):
                        nc.tensor.matmul(
                            oap,
                            wio_m[:, 2 * kp:2 * kp + 2,
                                  mo * 128:(mo + 1) * 128],
                            x_m[:, 2 * kp:2 * kp + 2, c * CH:(c + 1) * CH],
                            start=False, stop=(kp == KI // 2 - 1),
                            perf_mode=DR)
                    # 256-col halves pipeline sigmoid -> store, shrinking the
                    # post-matmul tail
                    for h in range(2):
                        hs = slice(h * CH // 2, (h + 1) * CH // 2)
                        o = opool.tile([128, CH // 2], BF16, tag="o", bufs=4,
                                       name="o")
                        nc.scalar.activation(o[:], oap[:, hs], AF.Sigmoid,
                                             bias=obc_t[:, mo:mo + 1],
                                             scale=INV)
                        eng = nc.sync if (2 * mo + h) % 2 == 0 else nc.scalar
                        eng.dma_start(
                            outT[mo * 128:(mo + 1) * 128,
                                 c * CH + h * CH // 2:
                                 c * CH + (h + 1) * CH // 2],
                            o[:])

            hh_step(1, N_STEPS - 2)
            out_chunk(0)
            out_chunk(1)

    nc.compile()
    return nc


_NC_CACHE = None


def _get_nc():
    global _NC_CACHE
    if _NC_CACHE is None:
        _NC_CACHE = _build_nc()
    return _NC_CACHE


def _make_bmask():
    m = np.zeros((128, 2 * CH), np.uint8)
    m[:_B1 - (_B1 // 128) * 128, 0:CH] = 1          # tile 5: parts < 43 tanh
    m[_B2 - (_B2 // 128) * 128:, CH:2 * CH] = 1     # tile 10: parts >= 86 relu
    return m


def _q8(w):
    """fp8 e4m3 quantize at the 256x weight scale."""
    return np.clip(np.asarray(w, np.float32) * SW, -240.0, 240.0).astype(E4)


def _prep_in_maps(inputs):
    x = np.asarray(inputs["inputs"], np.float32)
    hr = np.asarray(inputs["hidden_responses"], np.float32)[PERM]
    hb = np.asarray(inputs["hidden_biases"], np.float32)[PERM]
    orr = np.asarray(inputs["output_responses"], np.float32)
    ob = np.asarray(inputs["output_biases"], np.float32)

    wih_s = (hr[:, None] * np.asarray(inputs["input_to_hidden"], np.float32)[PERM]).T
    whh_s = (hr[:, None] *
             np.asarray(inputs["hidden_to_hidden"], np.float32)[PERM][:, PERM]).T
    who_s = (orr[:, None] *
             np.asarray(inputs["hidden_to_output"], np.float32)[:, PERM]).T
    wio_s = (orr[:, None] * np.asarray(inputs["input_to_output"], np.float32)).T

    def pack(w, ktiles):     # (ktiles*128, C) -> (128, ktiles, C)
        c = w.shape[1]
        return np.ascontiguousarray(
            w.reshape(ktiles, 128, c).transpose(1, 0, 2))

    # who: k-tile kk = 4j+sj lives at dim1 index kk (contiguous groups of 4)
    who_p = who_s.reshape(KO, KO, 128, NO).transpose(0, 2, 1, 3).reshape(NO, KO * NO)
    # whh: row-block J packs k-tiles 4J..4J+3
    whh_p = whh_s.reshape(4, 4, 128, NH).transpose(0, 2, 1, 3).reshape(4 * 128, 4, NH)

    shared = {
        "wih": _q8(pack(wih_s, KI)),
        "whh": np.ascontiguousarray(_q8(whh_p)),
        "who": _q8(pack(np.ascontiguousarray(who_p), KO)).reshape(128, KO * KO, NO),
        "wio": _q8(pack(wio_s, KI)),
        "hbc": np.ascontiguousarray(hb.reshape(KH, 128).T),
        "obc": np.ascontiguousarray(ob.reshape(KO, 128).T),
        "bmask": _make_bmask(),
    }
    in_maps = []
    for c in range(N_CORES):
        m = dict(shared)
        xq = np.clip(x[c * BL:(c + 1) * BL].T, -240.0, 240.0).astype(E4)
        m["xT"] = pack(np.ascontiguousarray(xq), KI)
        in_maps.append(m)
    return in_maps


def _run(inputs, trace=False, tmpdir=None):
    nc = _get_nc()
    in_maps = _prep_in_maps(inputs)
    res = run_bass_kernel_spmd(nc, in_maps, core_ids=list(range(N_CORES)),
                               trace=trace, tmpdir=tmpdir)
    out = np.empty((B, NO), np.float32)
    for c in range(N_CORES):
        out[c * BL:(c + 1) * BL] = res.results[c]["outT"].astype(np.float32).T
    return out, res


def kernel(**inputs) -> np.ndarray:
    out, _ = _run(inputs, trace=False)
    return out


if __name__ == "__main__":
    rng = np.random.default_rng(0)
    ins = {
        "inputs": rng.standard_normal((B, NI), dtype=np.float32),
        "input_to_hidden": rng.standard_normal((NH, NI), dtype=np.float32) * 0.02,
        "hidden_to_hidden": rng.standard_normal((NH, NH), dtype=np.float32) * 0.02,
        "output_to_hidden": rng.standard_normal((NH, NO), dtype=np.float32) * 0.02,
        "input_to_output": rng.standard_normal((NO, NI), dtype=np.float32) * 0.02,
        "hidden_to_output": rng.standard_normal((NO, NH), dtype=np.float32) * 0.02,
        "output_to_output": rng.standard_normal((NO, NO), dtype=np.float32) * 0.02,
        "hidden_responses": rng.standard_normal(NH, dtype=np.float32) * 0.1 + 1.0,
        "hidden_biases": rng.standard_normal(NH, dtype=np.float32) * 0.1,
        "output_responses": rng.standard_normal(NO, dtype=np.float32) * 0.1 + 1.0,
        "output_biases": rng.standard_normal(NO, dtype=np.float32) * 0.1,
    }
    out = kernel(**ins)
    print("kernel output", out.shape, out.dtype, out[:2, :4])


# revision 36
# speedup vs baseline: 1.0391x; 1.0391x over previous
"""Trainium2 Bass kernel for a 4-step differentiable recurrent net forward pass.

Reference computation (B=8192, NI=512, NH=2048, NO=512, 4 steps):
    activs = 0; outputs = 0
    repeat 4x:  pre = hr * (x @ Wih.T + activs @ Whh.T + outputs @ Woh.T) + hb
                activs = per_neuron_act(pre)        # tanh/sigmoid/relu by i%3
    out = sigmoid(or * (x @ Wio.T + outputs @ Woo.T + activs @ Who.T) + ob)

`outputs` is never written inside the loop, so the Woh/Woo terms vanish and
the x-projection P = hr*(x@Wih.T)+hb is loop-invariant (computed once).

Strategy: data-parallel on batch across 8 cores (1024 rows each). On-core
everything is feature-major (features on SBUF partitions, batch on the free
axis). All matmuls run in fp8 e4m3 with DoubleRow perf mode (two 128-deep
k-tiles contracted per pass -> 2x tensor throughput vs bf16); weights are
host-quantized at a x256 power-of-2 scale (so typical |w|~5 stays in e4m3's
normal range), activations/x are quantized at unit scale, and PSUM holds
256x the true (biasless) pre-activations; the 1/256 and the true hb/ob are
folded into the scalar-engine activation `scale`/`bias` operands. The
x-projection P' is evicted once per 4-bank PSUM block with a single DVE
read; the output layer accumulates x@Wio into the same PSUM group as
A@Who and applies sigmoid straight off PSUM. DMA dispatch order is tuned
from the perfetto trace: both HWDGE queues run 4-deep semaphore rings, so
pieces are ordered by consumption time and whh streams in 512KB halves.
Host-side prep: hidden neurons are permuted so the three activation groups
are contiguous, hr/or are folded into the weights, and weights are packed
so each piece loads as one contiguous-row DMA.
"""

import os

import numpy as np
import ml_dtypes

import concourse.bass as bass
import concourse.tile as tile
from concourse import bacc, mybir
from concourse.bass_utils import run_bass_kernel_spmd

B, NI, NH, NO = 8192, 512, 2048, 512
N_STEPS = 4
N_CORES = 8
BL = B // N_CORES          # batch rows per core
CH = 512                   # batch chunk (one PSUM bank of fp32)
NCH = BL // CH             # 2 chunks per core
KI = NI // 128             # 4 k-tiles over inputs
KH = NH // 128             # 16 k/m-tiles over hidden
KO = NO // 128             # 4 m-tiles over outputs

BF16 = mybir.dt.bfloat16
F32 = mybir.dt.float32
FP8 = mybir.dt.float8e4
E4 = ml_dtypes.float8_e4m3   # TRN-style e4m3, max normal 240
AF = mybir.ActivationFunctionType
DR = mybir.MatmulPerfMode.DoubleRow

SW = 256.0                 # weight quantization scale (power of 2)
INV = 1.0 / SW             # folded into activation-engine `scale`

# hidden neurons regrouped as [all tanh | all sigmoid | all relu]
_idx = np.arange(NH)
PERM = np.concatenate([_idx[_idx % 3 == 0], _idx[_idx % 3 == 1], _idx[_idx % 3 == 2]])
_B1 = int((_idx % 3 == 0).sum())           # 683
_B2 = _B1 + int((_idx % 3 == 1).sum())     # 1366

# per m-tile: the single activation function, or None for the two mixed tiles
_TILE_FUNC = []
for _m in range(KH):
    _lo, _hi = _m * 128, (_m + 1) * 128
    _fs = set()
    for _f, _a, _b in ((AF.Tanh, 0, _B1), (AF.Sigmoid, _B1, _B2), (AF.Relu, _B2, NH)):
        if max(_lo, _a) < min(_hi, _b):
            _fs.add(_f)
    _TILE_FUNC.append(_fs.pop() if len(_fs) == 1 else None)

# mixed tiles: (major_func applied everywhere, minor_func, mask column block)
# partition sub-ranges must be 32-aligned on TRN2, so the minority strip is
# fixed up with a full-tile ACT + copy_predicated against a {0,1} mask
_BOUNDARY = {
    _B1 // 128: (AF.Sigmoid, AF.Tanh, 0),    # tile 5: parts < 43 are tanh
    _B2 // 128: (AF.Sigmoid, AF.Relu, 1),    # tile 10: parts >= 86 are relu
}


def _emit_hidden_act(nc, ps, blk, a_new, tmp_pool, bmask_t, hbc_t):
    """Evict a 4-bank pre-activation block through the grouped activations.

    ps:    AP (128, 4*CH) holding 256x-scaled biasless pre-activations for
           m-tiles blk*4..blk*4+3 (SBUF f32/bf16); the true hidden bias is
           applied per-tile via the ACT bias operand
    a_new: SBUF tile (128, KH, CH) fp8, m-tile m lives at [:, m, :]
    """
    for mloc in range(4):
        m = blk * 4 + mloc
        bias = hbc_t[:, m:m + 1]
        if m in _BOUNDARY:
            major, minor, mb = _BOUNDARY[m]
            nc.scalar.activation(
                a_new[:, m, :],
                ps[:, mloc * CH:(mloc + 1) * CH], major, bias=bias, scale=INV)
            t = tmp_pool.tile([128, CH], FP8, tag="btmp", bufs=2, name="btmp")
            nc.scalar.activation(t[:], ps[:, mloc * CH:(mloc + 1) * CH], minor,
                                 bias=bias, scale=INV)
            nc.vector.copy_predicated(
                a_new[:, m, :],
                bmask_t[:, mb * CH:(mb + 1) * CH], t[:])
        else:
            nc.scalar.activation(
                a_new[:, m, :],
                ps[:, mloc * CH:(mloc + 1) * CH], _TILE_FUNC[m],
                bias=bias, scale=INV)


def _build_nc():
    nc = bacc.Bacc("TRN2", target_bir_lowering=False, debug=False,
                   num_devices=N_CORES, dynamic_dma_scratch_size=2048)

    # all operands host-packed so each loads as one large contiguous DMA:
    # [p, k, c] = W[k*128 + p, c] for k-tile k; fp8 e4m3 at 256x scale
    xT = nc.dram_tensor("xT", [128, KI, BL], FP8, kind="ExternalInput").ap()
    wih = nc.dram_tensor("wih", [128, KI, NH], FP8, kind="ExternalInput").ap()
    whh = nc.dram_tensor("whh", [4 * 128, 4, NH], FP8,
                         kind="ExternalInput").ap()
    who = nc.dram_tensor("who", [128, KO * KO, NO], FP8,
                         kind="ExternalInput").ap()
    wio = nc.dram_tensor("wio", [128, KI, NO], FP8, kind="ExternalInput").ap()
    hbc = nc.dram_tensor("hbc", [128, KH], F32, kind="ExternalInput").ap()
    obc = nc.dram_tensor("obc", [128, KO], F32, kind="ExternalInput").ap()
    bmask = nc.dram_tensor("bmask", [128, 2 * CH], mybir.dt.uint8,
                           kind="ExternalInput").ap()
    outT = nc.dram_tensor("outT", [NO, BL], BF16, kind="ExternalOutput").ap()

    with tile.TileContext(nc) as tc:
        with tc.tile_pool(name="w", bufs=1) as wpool, \
             tc.tile_pool(name="act", bufs=1) as apool, \
             tc.tile_pool(name="ps", bufs=2, space="PSUM") as pspool, \
             tc.tile_pool(name="out", bufs=4) as opool:

            # ---- stage inputs. DMA dispatch order is tuned from the trace:
            # each HWDGE queue (SP=sync, ACT=scalar) runs a 4-deep semaphore
            # ring, so pieces are ordered by consumption time and whh is
            # split into 512KB halves so the rings stream continuously.
            # The very first pieces are exactly what the first matmuls need
            # (k-pair 0/1, m-tiles 0-3 + x chunk 0). ----
            wih_m = wpool.tile([128, KI, NH], FP8, tag="projA", name="wihm")
            x_m = wpool.tile([128, KI, BL], FP8, tag="x", name="xm")
            # SP queue: wih in P-loop consumption order, tiny per-partition
            # tensors, then whh J0/J2 halves
            nc.sync.dma_start(wih_m[:, 0:2, 0:4 * 128], wih[:, 0:2, 0:4 * 128])
            nc.sync.dma_start(wih_m[:, 2:4, 0:4 * 128], wih[:, 2:4, 0:4 * 128])
            nc.sync.dma_start(wih_m[:, 0:4, 4 * 128:8 * 128],
                              wih[:, 0:4, 4 * 128:8 * 128])
            hbc_t = wpool.tile([128, KH], F32, tag="hbc")
            nc.sync.dma_start(hbc_t[:], hbc[:])
            bmask_t = wpool.tile([128, 2 * CH], mybir.dt.uint8, tag="bmask")
            nc.sync.dma_start(bmask_t[:], bmask[:])
            obc_t = wpool.tile([128, KO], F32, tag="obc")
            nc.sync.dma_start(obc_t[:], obc[:])
            # ACT queue: x and the wih tail interleaved by consumption time,
            # then whh J1/J3 halves
            nc.scalar.dma_start(x_m[:, 0:2, 0:CH], xT[:, 0:2, 0:CH])
            nc.scalar.dma_start(x_m[:, 2:4, 0:CH], xT[:, 2:4, 0:CH])
            nc.scalar.dma_start(wih_m[:, 0:4, 8 * 128:12 * 128],
                                wih[:, 0:4, 8 * 128:12 * 128])
            nc.scalar.dma_start(x_m[:, 0:2, CH:BL], xT[:, 0:2, CH:BL])
            nc.scalar.dma_start(wih_m[:, 0:4, 12 * 128:NH],
                                wih[:, 0:4, 12 * 128:NH])
            nc.scalar.dma_start(x_m[:, 2:4, CH:BL], xT[:, 2:4, CH:BL])
            whh_m = [wpool.tile([128, 4, NH], FP8, tag=f"whhJ{J}",
                                name=f"whhJ{J}") for J in range(4)]
            for J, eng in ((0, nc.sync), (1, nc.scalar),
                           (2, nc.sync), (3, nc.scalar)):
                for h in range(2):
                    eng.dma_start(whh_m[J][:, 2 * h:2 * h + 2, :],
                                  whh[J * 128:(J + 1) * 128, 2 * h:2 * h + 2, :])

            # ---- per-chunk x-projection P' (256x scaled, biasless) and
            # first-step activations ----
            P = {}
            A = {}

            def p_chunk(c):
                P[c] = apool.tile([128, KH * CH], BF16, tag=f"P{c}",
                                  name=f"P{c}")
                a1 = apool.tile([128, KH, CH], FP8, tag="A", bufs=3,
                                name=f"A1c{c}")
                for blk in range(4):
                    ps = pspool.tile([128, 4 * CH], F32, tag="ps", name="psb")
                    for kp in range(KI // 2):
                        for mloc in range(4):
                            m = blk * 4 + mloc
                            nc.tensor.matmul(
                                ps[:, mloc * CH:(mloc + 1) * CH],
                                wih_m[:, 2 * kp:2 * kp + 2,
                                      m * 128:(m + 1) * 128],
                                x_m[:, 2 * kp:2 * kp + 2,
                                    c * CH:(c + 1) * CH],
                                start=(kp == 0), stop=(kp == KI // 2 - 1),
                                perf_mode=DR)
                    # single DVE read frees the 4-bank PSUM slot; ACTs then
                    # run off SBUF with the bias applied per tile
                    nc.vector.tensor_copy(
                        P[c][:, blk * 4 * CH:(blk + 1) * 4 * CH], ps[:])
                    _emit_hidden_act(nc, P[c][:, blk * 4 * CH:(blk + 1) * 4 * CH],
                                     blk, a1, opool, bmask_t, hbc_t)
                A[c] = a1

            p_chunk(0)
            p_chunk(1)

            # ---- recurrent steps 2..4 ----
            def hh_step(c, s):
                a_new = apool.tile([128, KH, CH], FP8, tag="A", bufs=3,
                                   name=f"A{s + 2}c{c}")
                for blk in range(4):
                    ps = pspool.tile([128, 4 * CH], F32, tag="ps", name="psb")
                    for kp in range(KH // 2):
                        J, sub = divmod(kp, 2)
                        for mloc in range(4):
                            m = blk * 4 + mloc
                            nc.tensor.matmul(
                                ps[:, mloc * CH:(mloc + 1) * CH],
                                whh_m[J][:, 2 * sub:2 * sub + 2,
                                         m * 128:(m + 1) * 128],
                                A[c][:, 2 * kp:2 * kp + 2, :],
                                start=(kp == 0), stop=(kp == KH // 2 - 1),
                                perf_mode=DR)
                    # pre' = psum + P' into an SBUF temp: a single PSUM read
                    # frees the bank; ACT then runs off SBUF
                    tmp = opool.tile([128, 4 * CH], F32, tag="pre", bufs=3,
                                     name="pre")
                    nc.vector.tensor_add(
                        tmp[:], ps[:], P[c][:, blk * 4 * CH:(blk + 1) * 4 * CH])
                    _emit_hidden_act(nc, tmp, blk, a_new, opool, bmask_t, hbc_t)
                A[c] = a_new

            for s in range(N_STEPS - 2):
                for c in range(NCH):
                    hh_step(c, s)
            hh_step(0, N_STEPS - 2)  # chunk 1's final step emitted after who

            # ---- output layer (who reuses the wih slot); chunk 0's
            # output overlaps chunk 1's final hh step. The x@Wio term is
            # accumulated into the same PSUM group, and sigmoid runs
            # directly off PSUM with bias+scale — no outx staging at all ----
            wio_m = wpool.tile([128, KI, NO], FP8, tag="wio", name="wiom")
            nc.sync.dma_start(wio_m[:], wio[:])
            who_m = wpool.tile([128, KO * KO, NO], FP8, tag="projA",
                               name="whom")
            nc.sync.dma_start(who_m[:, 0:8, :], who[:, 0:8, :])
            nc.sync.dma_start(who_m[:, 8:16, :], who[:, 8:16, :])

            def out_chunk(c):
                for mo in range(KO):
                    pso = pspool.tile([128, CH], F32, tag="ps", name="pso")
                    oap = pso[:]
                    for t2 in range(KH // 2):
                        nc.tensor.matmul(
                            oap,
                            who_m[:, 2 * t2:2 * t2 + 2,
                                  mo * 128:(mo + 1) * 128],
                            A[c][:, 2 * t2:2 * t2 + 2, :],
                            start=(t2 == 0), stop=False, perf_mode=DR)
                    for kp in range(KI // 2):
                        nc.tensor.matmul(
                            oap,
                            wio_m[:, 2 * kp:2 * kp + 2,
                                  mo * 128:(mo + 1) * 128],
                            x_m[:, 2 * kp:2 * kp + 2, c * CH:(c + 1) * CH],
                            start=False, stop=(kp == KI // 2 - 1),
                            perf_mode=DR)
                    # 256-col halves pipeline sigmoid -> store, shrinking the
                    # post-matmul tail
                    for h in range(2):
                        hs = slice(h * CH // 2, (h + 1) * CH // 2)
                        o = opool.tile([128, CH // 2], BF16, tag="o", bufs=4,
                                       name="o")
                        nc.scalar.activation(o[:], oap[:, hs], AF.Sigmoid,
                                             bias=obc_t[:, mo:mo + 1],
                                             scale=INV)
                        eng = nc.sync if (2 * mo + h) % 2 == 0 else nc.scalar
                        eng.dma_start(
                            outT[mo * 128:(mo + 1) * 128,
                                 c * CH + h * CH // 2:
                                 c * CH + (h + 1) * CH // 2],
                            o[:])

            hh_step(1, N_STEPS - 2)
            out_chunk(0)
            out_chunk(1)

    nc.compile()
    return nc


_NC_CACHE = None


def _get_nc():
    global _NC_CACHE
    if _NC_CACHE is None:
        _NC_CACHE = _build_nc()
    return _NC_CACHE


def _make_bmask():
    m = np.zeros((128, 2 * CH), np.uint8)
    m[:_B1 - (_B1 // 128) * 128, 0:CH] = 1          # tile 5: parts < 43 tanh
    m[_B2 - (_B2 // 128) * 128:, CH:2 * CH] = 1     # tile 10: parts >= 86 relu
    return m


def _q8(w):
    """fp8 e4m3 quantize at the 256x weight scale."""
    return np.clip(np.asarray(w, np.float32) * SW, -240.0, 240.0).astype(E4)


def _prep_in_maps(inputs):
    x = np.asarray(inputs["inputs"], np.float32)
    hr = np.asarray(inputs["hidden_responses"], np.float32)[PERM]
    hb = np.asarray(inputs["hidden_biases"], np.float32)[PERM]
    orr = np.asarray(inputs["output_responses"], np.float32)
    ob = np.asarray(inputs["output_biases"], np.float32)

    wih_s = (hr[:, None] * np.asarray(inputs["input_to_hidden"], np.float32)[PERM]).T
    whh_s = (hr[:, None] *
             np.asarray(inputs["hidden_to_hidden"], np.float32)[PERM][:, PERM]).T
    who_s = (orr[:, None] *
             np.asarray(inputs["hidden_to_output"], np.float32)[:, PERM]).T
    wio_s = (orr[:, None] * np.asarray(inputs["input_to_output"], np.float32)).T

    def pack(w, ktiles):     # (ktiles*128, C) -> (128, ktiles, C)
        c = w.shape[1]
        return np.ascontiguousarray(
            w.reshape(ktiles, 128, c).transpose(1, 0, 2))

    # who: k-tile kk = 4j+sj lives at dim1 index kk (contiguous groups of 4)
    who_p = who_s.reshape(KO, KO, 128, NO).transpose(0, 2, 1, 3).reshape(NO, KO * NO)
    # whh: row-block J packs k-tiles 4J..4J+3
    whh_p = whh_s.reshape(4, 4, 128, NH).transpose(0, 2, 1, 3).reshape(4 * 128, 4, NH)

    shared = {
        "wih": _q8(pack(wih_s, KI)),
        "whh": np.ascontiguousarray(_q8(whh_p)),
        "who": _q8(pack(np.ascontiguousarray(who_p), KO)).reshape(128, KO * KO, NO),
        "wio": _q8(pack(wio_s, KI)),
        "hbc": np.ascontiguousarray(hb.reshape(KH, 128).T),
        "obc": np.ascontiguousarray(ob.reshape(KO, 128).T),
        "bmask": _make_bmask(),
    }
    in_maps = []
    for c in range(N_CORES):
        m = dict(shared)
        xq = np.clip(x[c * BL:(c + 1) * BL].T, -240.0, 240.0).astype(E4)
        m["xT"] = pack(np.ascontiguousarray(xq), KI)
        in_maps.append(m)
    return in_maps


def _run(inputs, trace=False, tmpdir=None):
    nc = _get_nc()
    in_maps = _prep_in_maps(inputs)
    res = run_bass_kernel_spmd(nc, in_maps, core_ids=list(range(N_CORES)),
                               trace=trace, tmpdir=tmpdir)
    out = np.empty((B, NO), np.float32)
    for c in range(N_CORES):
        out[c * BL:(c + 1) * BL] = res.results[c]["outT"].astype(np.float32).T
    return out, res


def kernel(**inputs) -> np.ndarray:
    out, _ = _run(inputs, trace=False)
    return out


if __name__ == "__main__":
    rng = np.random.default_rng(0)
    ins = {
        "inputs": rng.standard_normal((B, NI), dtype=np.float32),
        "input_to_hidden": rng.standard_normal((NH, NI), dtype=np.float32) * 0.02,
        "hidden_to_hidden": rng.standard_normal((NH, NH), dtype=np.float32) * 0.02,
        "output_to_hidden": rng.standard_normal((NH, NO), dtype=np.float32) * 0.02,
        "input_to_output": rng.standard_normal((NO, NI), dtype=np.float32) * 0.02,
        "hidden_to_output": rng.standard_normal((NO, NH), dtype=np.float32) * 0.02,
        "output_to_output": rng.standard_normal((NO, NO), dtype=np.float32) * 0.02,
        "hidden_responses": rng.standard_normal(NH, dtype=np.float32) * 0.1 + 1.0,
        "hidden_biases": rng.standard_normal(NH, dtype=np.float32) * 0.1,
        "output_responses": rng.standard_normal(NO, dtype=np.float32) * 0.1 + 1.0,
        "output_biases": rng.standard_normal(NO, dtype=np.float32) * 0.1,
    }
    out = kernel(**ins)
    print("kernel output", out.shape, out.dtype, out[:2, :4])
